# revision 5
# baseline (speedup 1.0000x reference)
"""GCNConv-pair (mu/logstd) message-passing kernel for 8 trn2 NeuronCores.

Strategy:
  - Host: fold sym-norm dinv into x rows; build per-core edge structures
    (edges partitioned by destination core, bucketed into 128-dst windows,
    split lo/hi by source id so gather indices fit int16, padded to uniform
    128-edge chunks per window across cores).
  - Device (SPMD, identical program, per-core data):
      Phase 1: hs = (dinv*x) @ [W_mu | W_logstd]  (bf16 matmul, full table
               replicated per core), written to DRAM.
      Phase 2: dma_gather of hs rows for each edge, one-hot chunk matmuls
               (TensorE) accumulate segment sums per 128-dst window in PSUM,
               fused epilogue out = psum * dinv_dst + bias, DMA to outputs.
"""

import os, sys

sys.path.insert(0, "/opt/trn_rl_repo")

import numpy as np
import ml_dtypes

import concourse.bass as bass
import concourse.bacc as bacc
import concourse.tile as tile
from concourse import mybir
from concourse.bass_utils import run_bass_kernel_spmd

# ---- problem constants (hardcoded per harness contract) ----
N_NODES = 50000
N_EDGES = 800000
IN_SIZE = 512
OUT_SIZE = 128
DOUT = 2 * OUT_SIZE  # 256, mu|logstd concatenated
NCORES = 8
NPC = N_NODES // NCORES  # 6250 nodes per core
NW = (NPC + 127) // 128  # 49 windows of 128 dst nodes
LASTW_ROWS = NPC - (NW - 1) * 128  # 106
SPLIT = 25000  # lo/hi source split for int16 gather indices
NPAD = 50176  # node count padded to 1024*49 for matmul blocking
G_WIN = 2  # windows per gather tile

BF16 = ml_dtypes.bfloat16
LAST_RESULT = None


def _install_ntff_shim():
    """Register the axon NTFF profile hook if the glue module is missing."""
    try:
        import contextlib
        import ctypes
        import types

        import antenv  # noqa: F401

        if "antenv.axon_hooks" in sys.modules:
            return
        so_path = "/opt/axon/libaxon_pjrt.so"
        try:
            lib = ctypes.CDLL(so_path)
        except OSError:
            return
        if not hasattr(lib, "axon_start_nrt_profile"):
            return
        lib.axon_start_nrt_profile.argtypes = [
            ctypes.POINTER(ctypes.c_int64),
            ctypes.c_size_t,
        ]
        lib.axon_start_nrt_profile.restype = ctypes.c_int64
        lib.axon_stop_nrt_profile.argtypes = [ctypes.c_char_p]
        lib.axon_stop_nrt_profile.restype = ctypes.c_int64

        @contextlib.contextmanager
        def _hook(output_dir, device_ids):
            import jax

            jax.devices()
            if device_ids:
                ids = (ctypes.c_int64 * len(device_ids))(*device_ids)
                rc = lib.axon_start_nrt_profile(ids, len(device_ids))
            else:
                rc = lib.axon_start_nrt_profile(None, 0)
            if rc != 0:
                raise RuntimeError(f"axon_start_nrt_profile rc={rc}")
            try:
                yield
            finally:
                n = lib.axon_stop_nrt_profile(str(output_dir).encode())
                if n < 0:
                    raise RuntimeError(f"axon_stop_nrt_profile rc={n}")

        hook = _hook
        mod = types.ModuleType("antenv.axon_hooks")
        mod.set_axon_ntff_profile_hook = lambda h: None
        mod.get_axon_ntff_profile_hook = lambda: hook
        sys.modules["antenv.axon_hooks"] = mod
        antenv.axon_hooks = mod
    except Exception:
        pass


def _wrap_idx16(idx_flat):
    """[S] int array -> [128, S/16] int16 wrapped+replicated layout."""
    assert idx_flat.size % 16 == 0
    w = idx_flat.reshape(-1, 16).T.astype(np.int16)  # [16, S/16]
    return np.tile(w, (8, 1))  # [128, S/16]


def _prep_edges(edge_index):
    """Host-side edge partitioning. Returns per-core gather structures."""
    src = np.asarray(edge_index[0], dtype=np.int64)
    dst = np.asarray(edge_index[1], dtype=np.int64)
    loops = np.arange(N_NODES, dtype=np.int64)
    src = np.concatenate([src, loops])
    dst = np.concatenate([dst, loops])

    deg = np.bincount(dst, minlength=N_NODES).astype(np.float64)
    dinv = (1.0 / np.sqrt(np.maximum(deg, 1.0))).astype(np.float32)

    core = dst // NPC
    dstl = dst - core * NPC
    win = dstl >> 7
    slot = (dstl & 127).astype(np.float32)
    view = (src >= SPLIT).astype(np.int64)
    idx16 = np.where(view == 0, src, src - SPLIT)

    # counts[c, v, w]
    counts = np.zeros((NCORES, 2, NW), np.int64)
    flat_key = (core * 2 + view) * NW + win
    cnt = np.bincount(flat_key, minlength=NCORES * 2 * NW)
    counts[:] = cnt.reshape(NCORES, 2, NW)

    # uniform chunks-per-window across cores+views
    kw = np.maximum(1, (counts.max(axis=(0, 1)) + 127) // 128)  # [NW]
    chunk_off = np.concatenate([[0], np.cumsum(kw)]).astype(np.int64)  # [NW+1]
    n_chunks = int(chunk_off[-1])
    S = n_chunks * 128  # padded slots per (core, view)

    idx_arrs = np.zeros((NCORES, 2, 128, S // 16), np.int16)
    slot_arrs = np.full((NCORES, 2, 128, n_chunks), -1.0, np.float32)

    # position of each edge inside its (core, view, window) bucket
    order = np.lexsort((win, view, core))
    s_src = idx16[order]
    s_slot = slot[order]
    s_key = flat_key[order]
    # rank within each bucket
    bucket_start = np.zeros(NCORES * 2 * NW + 1, np.int64)
    bucket_start[1:] = np.cumsum(cnt)
    rank = np.arange(src.size) - bucket_start[s_key]
    s_core = s_key // (2 * NW)
    s_view = (s_key // NW) % 2
    s_win = s_key % NW
    pos = chunk_off[s_win] * 128 + rank  # position in padded slot array

    for c in range(NCORES):
        for v in range(2):
            m = (s_core == c) & (s_view == v)
            idx_flat = np.zeros(S, np.int64)
            slot_flat = np.full(S, -1.0, np.float32)
            idx_flat[pos[m]] = s_src[m]
            slot_flat[pos[m]] = s_slot[m]
            idx_arrs[c, v] = _wrap_idx16(idx_flat)
            slot_arrs[c, v] = slot_flat.reshape(-1, 128).T

    dinv_out = np.zeros((NCORES, 128, NW), np.float32)
    for c in range(NCORES):
        d = np.zeros(NW * 128, np.float32)
        d[:NPC] = dinv[c * NPC : (c + 1) * NPC]
        dinv_out[c] = d.reshape(NW, 128).T

    return {
        "dinv": dinv,
        "kw": kw.astype(np.int64),
        "chunk_off": chunk_off,
        "n_chunks": n_chunks,
        "idx_arrs": idx_arrs,
        "slot_arrs": slot_arrs.astype(BF16),
        "dinv_out": dinv_out,
    }


def _build_program(kw, chunk_off, n_chunks):
    """Build the SPMD bass program (identical across cores)."""
    nc = bacc.Bacc(
        "TRN2", target_bir_lowering=False, debug=False, num_devices=NCORES
    )
    bf16 = mybir.dt.bfloat16
    f32 = mybir.dt.float32
    i16 = mybir.dt.int16
    S = n_chunks * 128

    t_xsT = nc.dram_tensor("xsT", [4, 128, NPAD], bf16, kind="ExternalInput")
    t_w = nc.dram_tensor("wcat", [4, 128, DOUT], bf16, kind="ExternalInput")
    t_bias = nc.dram_tensor("bias", [128, DOUT], f32, kind="ExternalInput")
    t_iota = nc.dram_tensor("iota", [128, 128], bf16, kind="ExternalInput")
    t_idx = nc.dram_tensor("idx", [2, 128, S // 16], i16, kind="ExternalInput")
    t_slot = nc.dram_tensor("slot", [2, 128, n_chunks], bf16, kind="ExternalInput")
    t_dinv = nc.dram_tensor("dinv_out", [128, NW], f32, kind="ExternalInput")
    t_mu = nc.dram_tensor("out_mu", [NPC, OUT_SIZE], f32, kind="ExternalOutput")
    t_ls = nc.dram_tensor("out_ls", [NPC, OUT_SIZE], f32, kind="ExternalOutput")

    with tile.TileContext(nc) as tc:
        with (
            tc.tile_pool(name="dram", bufs=1, space="DRAM") as dram,
            tc.tile_pool(name="const", bufs=1) as const,
            tc.tile_pool(name="xk", bufs=3) as xkp,
            tc.tile_pool(name="hst", bufs=4) as hstp,
            tc.tile_pool(name="msg", bufs=2) as msgp,
            tc.tile_pool(name="oh", bufs=2) as ohp,
            tc.tile_pool(name="small", bufs=2) as smallp,
            tc.tile_pool(name="res", bufs=4) as resp,
            tc.tile_pool(name="psum", bufs=4, space="PSUM") as psp,
        ):
            hs_full = dram.tile([NPAD, DOUT], bf16)

            # constants
            w_tiles = []
            for kt in range(4):
                wt = const.tile([128, DOUT], bf16, name=f"w{kt}", tag=f"w{kt}")
                nc.sync.dma_start(wt[:], t_w[kt])
                w_tiles.append(wt)
            bias_t = const.tile([128, DOUT], f32, tag="bias")
            nc.sync.dma_start(bias_t[:], t_bias[:])
            iota_t = const.tile([128, 128], bf16, tag="iota")
            nc.sync.dma_start(iota_t[:], t_iota[:])
            dinv_t = const.tile([128, NW], f32, tag="dinv")
            nc.sync.dma_start(dinv_t[:], t_dinv[:])

            # ---- Phase 1: hs = xs @ Wcat (replicated over full table) ----
            for sb in range(NPAD // 1024):
                xk = [None] * 4
                for kt in range(4):
                    xk[kt] = xkp.tile([128, 1024], bf16, name=f"xk{kt}", tag=f"xk{kt}")
                    nc.sync.dma_start(
                        xk[kt][:], t_xsT[kt, :, sb * 1024 : (sb + 1) * 1024]
                    )
                for nb in range(8):
                    ph = psp.tile([128, DOUT], f32, tag="ph")
                    for kt in range(4):
                        nc.tensor.matmul(
                            out=ph[:],
                            lhsT=xk[kt][:, nb * 128 : (nb + 1) * 128],
                            rhs=w_tiles[kt][:],
                            start=(kt == 0),
                            stop=(kt == 3),
                        )
                    hstage = hstp.tile([128, DOUT], bf16, tag="hstage")
                    nc.vector.tensor_copy(hstage[:], ph[:])
                    r0 = (sb * 8 + nb) * 128
                    nc.sync.dma_start(hs_full[r0 : r0 + 128, :], hstage[:])

            # gather source views (row tables)
            hs_ap = hs_full[:]
            view_lo = hs_ap[0:32768, :]
            view_hi = hs_ap[SPLIT:NPAD, :]

            # ---- Phase 2: gather + one-hot segment-sum ----
            for w0 in range(0, NW, G_WIN):
                w1 = min(w0 + G_WIN, NW)
                ch0, ch1 = int(chunk_off[w0]), int(chunk_off[w1])
                ch = ch1 - ch0  # chunks in this tile
                n_idx = ch * 128
                msg = [None, None]
                oh = [None, None]
                for v in range(2):
                    idx_t = smallp.tile([128, n_idx // 16], i16, tag=f"idx{v}")
                    nc.sync.dma_start(
                        idx_t[:], t_idx[v, :, ch0 * 8 : ch1 * 8]
                    )
                    slot_t = smallp.tile([128, ch], bf16, tag=f"slot{v}")
                    nc.sync.dma_start(slot_t[:], t_slot[v, :, ch0:ch1])
                    msg[v] = msgp.tile([128, ch, DOUT], bf16, name=f"msg{v}", tag=f"msg{v}")
                    if os.environ.get("K_NO_GATHER"):
                        nc.vector.memset(msg[v][:], 0.0)
                    else:
                        nc.gpsimd.dma_gather(
                            out_ap=msg[v][:],
                            in_ap=view_lo if v == 0 else view_hi,
                            idxs_ap=idx_t[:],
                            num_idxs=n_idx,
                            num_idxs_reg=n_idx,
                            elem_size=DOUT,
                            single_packet=False,
                        )
                    oh[v] = ohp.tile([128, ch, 128], bf16, name=f"oh{v}", tag=f"oh{v}")
                    nc.vector.tensor_tensor(
                        out=oh[v][:],
                        in0=slot_t[:].unsqueeze(-1).broadcast_to([128, ch, 128]),
                        in1=iota_t[:].unsqueeze(1).broadcast_to([128, ch, 128]),
                        op=mybir.AluOpType.is_equal,
                    )
                for w in range(w0, w1):
                    k0 = int(chunk_off[w]) - ch0
                    k1 = int(chunk_off[w + 1]) - ch0
                    po = psp.tile([128, DOUT], f32, tag="po")
                    nmm = 2 * (k1 - k0)
                    i = 0
                    for v in range(2):
                        for k in range(k0, k1):
                            nc.tensor.matmul(
                                out=po[:],
                                lhsT=oh[v][:, k, :],
                                rhs=msg[v][:, k, :],
                                start=(i == 0),
                                stop=(i == nmm - 1),
                            )
                            i += 1
                    res = resp.tile([128, DOUT], f32, tag="res")
                    nc.vector.scalar_tensor_tensor(
                        out=res[:],
                        in0=po[:],
                        scalar=dinv_t[:, w : w + 1],
                        in1=bias_t[:],
                        op0=mybir.AluOpType.mult,
                        op1=mybir.AluOpType.add,
                    )
                    rows = LASTW_ROWS if w == NW - 1 else 128
                    r0 = w * 128
                    nc.sync.dma_start(
                        t_mu[r0 : r0 + rows, :], res[:rows, 0:OUT_SIZE]
                    )
                    nc.sync.dma_start(
                        t_ls[r0 : r0 + rows, :], res[:rows, OUT_SIZE:DOUT]
                    )

    nc.compile()
    return nc


def kernel(x, edge_index, W_mu, b_mu, W_logstd, b_logstd):
    _install_ntff_shim()

    x = np.asarray(x, dtype=np.float32)
    prep = _prep_edges(np.asarray(edge_index))

    # fold dinv into x rows; transpose + pad for matmul weight loading
    xs = (x * prep["dinv"][:, None]).astype(BF16)
    xsT = np.zeros((IN_SIZE, NPAD), BF16)
    xsT[:, :N_NODES] = xs.T
    xsT_t = np.ascontiguousarray(xsT.reshape(4, 128, NPAD))

    wcat = np.concatenate(
        [np.asarray(W_mu, np.float32), np.asarray(W_logstd, np.float32)], axis=1
    ).astype(BF16)
    wcat_t = np.ascontiguousarray(wcat.reshape(4, 128, DOUT))
    bias = np.concatenate(
        [np.asarray(b_mu, np.float32), np.asarray(b_logstd, np.float32)]
    ).astype(np.float32)
    bias_rep = np.tile(bias[None, :], (128, 1))
    iota_arr = np.tile(
        np.arange(128, dtype=np.float32).astype(BF16)[None, :], (128, 1)
    )

    nc = _build_program(prep["kw"], prep["chunk_off"], prep["n_chunks"])

    in_maps = []
    for c in range(NCORES):
        in_maps.append(
            {
                "xsT": xsT_t,
                "wcat": wcat_t,
                "bias": bias_rep,
                "iota": iota_arr,
                "idx": prep["idx_arrs"][c],
                "slot": prep["slot_arrs"][c],
                "dinv_out": prep["dinv_out"][c],
            }
        )

    trace = bool(os.environ.get("K_TRACE"))
    res = run_bass_kernel_spmd(
        nc, in_maps, core_ids=list(range(NCORES)), trace=trace
    )
    global LAST_RESULT
    LAST_RESULT = res
    if trace and res.exec_time_ns is not None:
        print(f"HW exec time: {res.exec_time_ns} ns")
    mu = np.concatenate([res.results[c]["out_mu"] for c in range(NCORES)], axis=0)
    ls = np.concatenate([res.results[c]["out_ls"] for c in range(NCORES)], axis=0)
    return (mu, ls)


# revision 9
# speedup vs baseline: 1.5127x; 1.5127x over previous
"""GCNConv-pair (mu/logstd) message-passing kernel for 8 trn2 NeuronCores.

Strategy:
  - Host: fold sym-norm dinv into x rows; partition edges by destination
    core; bucket into 128-dst windows; split lo/hi by (padded) source row
    so gather indices fit int16; pad per (window, view) to the max count
    across cores so the SPMD program is uniform.
  - Device (SPMD, identical program, per-core data):
      Phase 1: hs_slice = (dinv*x_slice) @ [W_mu | W_logstd] (bf16 matmul
               over this core's 6250 nodes), AllGather -> full hs table.
      Phase 2: per (window, view) dma_gather of hs rows (4 SWDGE queues in
               parallel), one-hot chunk matmuls (TensorE) accumulate the
               segment sum per 128-dst window in PSUM, fused epilogue
               out = psum * dinv_dst + bias, DMA to outputs.
"""

import os
import sys

sys.path.insert(0, "/opt/trn_rl_repo")

import numpy as np
import ml_dtypes

import concourse.bass as bass
import concourse.bacc as bacc
import concourse.tile as tile
from concourse import mybir
from concourse.bass_utils import run_bass_kernel_spmd

# ---- problem constants (hardcoded per harness contract) ----
N_NODES = 50000
N_EDGES = 800000
IN_SIZE = 512
OUT_SIZE = 128
DOUT = 2 * OUT_SIZE  # 256, mu|logstd concatenated
NCORES = 8
NPC = N_NODES // NCORES  # 6250 nodes per core
NW = (NPC + 127) // 128  # 49 windows of 128 dst nodes
LASTW_ROWS = NPC - (NW - 1) * 128  # 106
NPC_PAD = NW * 128  # 6272 padded rows per core in the hs table
NPAD = NPC_PAD * NCORES  # 50176
SPLIT = 25088  # lo/hi padded-row split for int16 gather indices
G_WIN = 4  # windows per one-hot build group
NQ = 4  # SWDGE queues used round-robin for gathers

BF16 = ml_dtypes.bfloat16
LAST_RESULT = None


def _install_ntff_shim():
    """Register the axon NTFF profile hook if the glue module is missing."""
    try:
        import contextlib
        import ctypes
        import types

        import antenv  # noqa: F401

        if "antenv.axon_hooks" in sys.modules:
            return
        so_path = "/opt/axon/libaxon_pjrt.so"
        try:
            lib = ctypes.CDLL(so_path)
        except OSError:
            return
        if not hasattr(lib, "axon_start_nrt_profile"):
            return
        lib.axon_start_nrt_profile.argtypes = [
            ctypes.POINTER(ctypes.c_int64),
            ctypes.c_size_t,
        ]
        lib.axon_start_nrt_profile.restype = ctypes.c_int64
        lib.axon_stop_nrt_profile.argtypes = [ctypes.c_char_p]
        lib.axon_stop_nrt_profile.restype = ctypes.c_int64

        @contextlib.contextmanager
        def _hook(output_dir, device_ids):
            import jax

            jax.devices()
            if device_ids:
                ids = (ctypes.c_int64 * len(device_ids))(*device_ids)
                rc = lib.axon_start_nrt_profile(ids, len(device_ids))
            else:
                rc = lib.axon_start_nrt_profile(None, 0)
            if rc != 0:
                raise RuntimeError(f"axon_start_nrt_profile rc={rc}")
            try:
                yield
            finally:
                n = lib.axon_stop_nrt_profile(str(output_dir).encode())
                if n < 0:
                    raise RuntimeError(f"axon_stop_nrt_profile rc={n}")

        hook = _hook
        mod = types.ModuleType("antenv.axon_hooks")
        mod.set_axon_ntff_profile_hook = lambda h: None
        mod.get_axon_ntff_profile_hook = lambda: hook
        sys.modules["antenv.axon_hooks"] = mod
        antenv.axon_hooks = mod
    except Exception:
        pass


def _prep_edges(edge_index):
    """Host-side edge partitioning. Returns per-core gather structures."""
    src = np.asarray(edge_index[0], dtype=np.int64)
    dst = np.asarray(edge_index[1], dtype=np.int64)
    loops = np.arange(N_NODES, dtype=np.int64)
    src = np.concatenate([src, loops])
    dst = np.concatenate([dst, loops])

    deg = np.bincount(dst, minlength=N_NODES).astype(np.float64)
    dinv = (1.0 / np.sqrt(np.maximum(deg, 1.0))).astype(np.float32)

    core = dst // NPC
    dstl = dst - core * NPC
    win = dstl >> 7
    slot = (dstl & 127).astype(np.float32)
    # padded hs-table row of the source node
    srow = src + 22 * (src // NPC)
    view = (srow >= SPLIT).astype(np.int64)
    idx16 = np.where(view == 0, srow, srow - SPLIT)

    flat_key = (core * 2 + view) * NW + win
    cnt = np.bincount(flat_key, minlength=NCORES * 2 * NW)
    counts = cnt.reshape(NCORES, 2, NW)

    # per (view, window) gather length: max count across cores
    rwv = counts.max(axis=0)  # [2, NW]
    kwv = (rwv + 127) // 128  # chunks per (view, window)
    # chunk offsets per view (for slot arrays / matmul indexing)
    chunk_off = np.zeros((2, NW + 1), np.int64)
    chunk_off[:, 1:] = np.cumsum(kwv, axis=1)
    n_chunks = chunk_off[:, -1]  # per view
    # idx column offsets per view: full chunks (128 idx = 8 cols of 16)
    col_off = chunk_off * 8
    n_cols = col_off[:, -1]

    idx_arrs = [
        np.zeros((NCORES, 128, int(n_cols[v])), np.int16) for v in range(2)
    ]
    slot_arrs = [
        np.full((NCORES, 128, int(n_chunks[v])), -1.0, np.float32)
        for v in range(2)
    ]

    order = np.lexsort((win, view, core))
    s_src = idx16[order]
    s_slot = slot[order]
    s_key = flat_key[order]
    bucket_start = np.zeros(NCORES * 2 * NW + 1, np.int64)
    bucket_start[1:] = np.cumsum(cnt)
    rank = np.arange(src.size) - bucket_start[s_key]
    s_core = s_key // (2 * NW)
    s_view = (s_key // NW) % 2
    s_win = s_key % NW

    for v in range(2):
        S_idx = int(n_cols[v]) * 16
        S_slot = int(n_chunks[v]) * 128
        idxpos = chunk_off[v][s_win] * 128 + rank
        slotpos = idxpos
        for c in range(NCORES):
            m = (s_core == c) & (s_view == v)
            idx_flat = np.zeros(S_idx, np.int64)
            idx_flat[idxpos[m]] = s_src[m]
            # wrapped: idx i of a window lives at [i%16, coloff + i//16];
            # since window offsets are 16-aligned this is a global reshape
            w16 = idx_flat.reshape(-1, 16).T.astype(np.int16)  # [16, S/16]
            idx_arrs[v][c] = np.tile(w16, (8, 1))
            slot_flat = np.full(S_slot, -1.0, np.float32)
            slot_flat[slotpos[m]] = s_slot[m]
            slot_arrs[v][c] = slot_flat.reshape(-1, 128).T

    dinv_out = np.zeros((NCORES, 128, NW), np.float32)
    for c in range(NCORES):
        d = np.zeros(NW * 128, np.float32)
        d[:NPC] = dinv[c * NPC : (c + 1) * NPC]
        dinv_out[c] = d.reshape(NW, 128).T

    return {
        "dinv": dinv,
        "rwv": rwv,
        "kwv": kwv,
        "chunk_off": chunk_off,
        "col_off": col_off,
        "n_chunks": n_chunks,
        "n_cols": n_cols,
        "idx_arrs": [a.astype(np.int16) for a in idx_arrs],
        "slot_arrs": [a.astype(BF16) for a in slot_arrs],
        "dinv_out": dinv_out,
    }


def _build_program(prep):
    """Build the SPMD bass program (identical across cores)."""
    rwv = prep["rwv"]
    kwv = prep["kwv"]
    chunk_off = prep["chunk_off"]
    col_off = prep["col_off"]
    n_chunks = prep["n_chunks"]
    n_cols = prep["n_cols"]

    nc = bacc.Bacc(
        "TRN2",
        target_bir_lowering=False,
        debug=False,
        num_devices=NCORES,
        num_swdge_queues=NQ,
    )
    bf16 = mybir.dt.bfloat16
    f32 = mybir.dt.float32
    i16 = mybir.dt.int16

    t_xsT = nc.dram_tensor("xsT", [4, 128, NPC_PAD], bf16, kind="ExternalInput")
    t_w = nc.dram_tensor("wcat", [4, 128, DOUT], bf16, kind="ExternalInput")
    t_bias = nc.dram_tensor("bias", [128, DOUT], f32, kind="ExternalInput")
    t_iota = nc.dram_tensor("iota", [128, 128], bf16, kind="ExternalInput")
    t_idx0 = nc.dram_tensor("idx0", [128, int(n_cols[0])], i16, kind="ExternalInput")
    t_idx1 = nc.dram_tensor("idx1", [128, int(n_cols[1])], i16, kind="ExternalInput")
    t_slot0 = nc.dram_tensor("slot0", [128, int(n_chunks[0])], bf16, kind="ExternalInput")
    t_slot1 = nc.dram_tensor("slot1", [128, int(n_chunks[1])], bf16, kind="ExternalInput")
    t_dinv = nc.dram_tensor("dinv_out", [128, NW], f32, kind="ExternalInput")
    t_mu = nc.dram_tensor("out_mu", [NPC, OUT_SIZE], f32, kind="ExternalOutput")
    t_ls = nc.dram_tensor("out_ls", [NPC, OUT_SIZE], f32, kind="ExternalOutput")
    t_idx = [t_idx0, t_idx1]
    t_slot = [t_slot0, t_slot1]

    with tile.TileContext(nc) as tc:
        with (
            tc.tile_pool(name="dram", bufs=1, space="DRAM") as dram,
            tc.tile_pool(name="const", bufs=1) as const,
            tc.tile_pool(name="hst", bufs=4) as hstp,
            tc.tile_pool(name="msg", bufs=5) as msgp,
            tc.tile_pool(name="oh", bufs=2) as ohp,
            tc.tile_pool(name="small", bufs=3) as smallp,
            tc.tile_pool(name="res", bufs=4) as resp,
            tc.tile_pool(name="psum", bufs=4, space="PSUM") as psp,
        ):
            hs_bounce = dram.tile([NPC_PAD, DOUT], bf16)
            hs_full = dram.tile([NPAD, DOUT], bf16, name="hs_full", addr_space="Shared")

            # constants
            xk = []
            for kt in range(4):
                xt_ = const.tile([128, NPC_PAD], bf16, name=f"xk{kt}")
                nc.sync.dma_start(xt_[:], t_xsT[kt])
                xk.append(xt_)
            w_tiles = []
            for kt in range(4):
                wt = const.tile([128, DOUT], bf16, name=f"w{kt}")
                nc.sync.dma_start(wt[:], t_w[kt])
                w_tiles.append(wt)
            bias_t = const.tile([128, DOUT], f32, name="bias_t")
            nc.sync.dma_start(bias_t[:], t_bias[:])
            iota_t = const.tile([128, 128], bf16, name="iota_t")
            nc.sync.dma_start(iota_t[:], t_iota[:])
            dinv_t = const.tile([128, NW], f32, name="dinv_t")
            nc.sync.dma_start(dinv_t[:], t_dinv[:])

            # ---- Phase 1: hs_slice = xs @ Wcat, then AllGather ----
            for g in range(0, NW, 4):
                gn = min(4, NW - g)
                hstage = hstp.tile([128, 4, DOUT], bf16, name="hstage")
                for j in range(gn):
                    nb = g + j
                    ph = psp.tile([128, DOUT], f32, name="ph", tag="ph")
                    for kt in range(4):
                        nc.tensor.matmul(
                            out=ph[:],
                            lhsT=xk[kt][:, nb * 128 : (nb + 1) * 128],
                            rhs=w_tiles[kt][:],
                            start=(kt == 0),
                            stop=(kt == 3),
                        )
                    nc.vector.tensor_copy(hstage[:, j, :], ph[:])
                nc.sync.dma_start(
                    hs_bounce[:]
                    .rearrange("(a p) d -> p a d", p=128)[:, g : g + gn, :],
                    hstage[:, :gn, :],
                )
            # NOTE: hs_bounce rows here are laid out [p, a, d] with node
            # = a*128 + p ... must match gather row indexing (node-major).
            nc.gpsimd.collective_compute(
                "AllGather",
                mybir.AluOpType.bypass,
                replica_groups=[list(range(NCORES))],
                ins=[hs_bounce[:].opt()],
                outs=[hs_full[:].opt()],
            )

            hs_ap = hs_full[:]
            views = [hs_ap[0:SPLIT + 128, :], hs_ap[SPLIT:NPAD, :]]

            # ---- Phase 2: gather + one-hot segment-sum ----
            qn = 0
            for g0 in range(0, NW, G_WIN):
                g1 = min(g0 + G_WIN, NW)
                oh = [None, None]
                msgs = {}
                for v in range(2):
                    ch0, ch1 = int(chunk_off[v][g0]), int(chunk_off[v][g1])
                    ch = ch1 - ch0
                    slot_t = smallp.tile([128, ch], bf16, name=f"slot{v}", tag=f"slot{v}")
                    nc.sync.dma_start(slot_t[:], t_slot[v][:, ch0:ch1])
                    oh[v] = ohp.tile([128, ch, 128], bf16, name=f"oh{v}", tag=f"oh{v}")
                    nc.vector.tensor_tensor(
                        out=oh[v][:],
                        in0=slot_t[:].unsqueeze(-1).broadcast_to([128, ch, 128]),
                        in1=iota_t[:].unsqueeze(1).broadcast_to([128, ch, 128]),
                        op=mybir.AluOpType.is_equal,
                    )
                    for w in range(g0, g1):
                        K = int(kwv[v][w])
                        co0, co1 = int(col_off[v][w]), int(col_off[v][w + 1])
                        idx_t = smallp.tile(
                            [128, co1 - co0], i16, name=f"idx{v}", tag=f"idx{v}"
                        )
                        nc.sync.dma_start(idx_t[:], t_idx[v][:, co0:co1])
                        kmax = int(kwv[v].max())
                        mt = msgp.tile(
                            [128, kmax, DOUT], bf16, name=f"msg{v}", tag=f"msg{v}"
                        )
                        nc.gpsimd.dma_gather(
                            out_ap=mt[:, :K, :],
                            in_ap=views[v],
                            idxs_ap=idx_t[:],
                            num_idxs=K * 128,
                            num_idxs_reg=K * 128,
                            elem_size=DOUT,
                            single_packet=False,
                            queue_num=qn,
                        )
                        qn = (qn + 1) % NQ
                        msgs[(v, w)] = mt
                for w in range(g0, g1):
                    po = psp.tile([128, DOUT], f32, name="po", tag="po")
                    nmm = int(kwv[0][w] + kwv[1][w])
                    i = 0
                    for v in range(2):
                        kbase = int(chunk_off[v][w]) - int(chunk_off[v][g0])
                        for k in range(int(kwv[v][w])):
                            nc.tensor.matmul(
                                out=po[:],
                                lhsT=oh[v][:, kbase + k, :],
                                rhs=msgs[(v, w)][:, k, :],
                                start=(i == 0),
                                stop=(i == nmm - 1),
                            )
                            i += 1
                    res = resp.tile([128, DOUT], f32, name="res", tag="res")
                    nc.vector.scalar_tensor_tensor(
                        out=res[:],
                        in0=po[:],
                        scalar=dinv_t[:, w : w + 1],
                        in1=bias_t[:],
                        op0=mybir.AluOpType.mult,
                        op1=mybir.AluOpType.add,
                    )
                    rows = LASTW_ROWS if w == NW - 1 else 128
                    r0 = w * 128
                    nc.sync.dma_start(t_mu[r0 : r0 + rows, :], res[:rows, 0:OUT_SIZE])
                    nc.sync.dma_start(
                        t_ls[r0 : r0 + rows, :], res[:rows, OUT_SIZE:DOUT]
                    )

    nc.compile()
    return nc


def kernel(x, edge_index, W_mu, b_mu, W_logstd, b_logstd):
    _install_ntff_shim()

    x = np.asarray(x, dtype=np.float32)
    prep = _prep_edges(np.asarray(edge_index))

    # fold dinv into x rows; per-core transposed slice [4,128,NPC_PAD]
    xs = (x * prep["dinv"][:, None]).astype(BF16)
    xsT_cores = []
    for c in range(NCORES):
        sl = np.zeros((IN_SIZE, NPC_PAD), BF16)
        sl[:, :NPC] = xs[c * NPC : (c + 1) * NPC].T
        xsT_cores.append(np.ascontiguousarray(sl.reshape(4, 128, NPC_PAD)))

    wcat = np.concatenate(
        [np.asarray(W_mu, np.float32), np.asarray(W_logstd, np.float32)], axis=1
    ).astype(BF16)
    wcat_t = np.ascontiguousarray(wcat.reshape(4, 128, DOUT))
    bias = np.concatenate(
        [np.asarray(b_mu, np.float32), np.asarray(b_logstd, np.float32)]
    ).astype(np.float32)
    bias_rep = np.tile(bias[None, :], (128, 1))
    iota_arr = np.tile(
        np.arange(128, dtype=np.float32).astype(BF16)[None, :], (128, 1)
    )

    nc = _build_program(prep)

    in_maps = []
    for c in range(NCORES):
        in_maps.append(
            {
                "xsT": xsT_cores[c],
                "wcat": wcat_t,
                "bias": bias_rep,
                "iota": iota_arr,
                "idx0": prep["idx_arrs"][0][c],
                "idx1": prep["idx_arrs"][1][c],
                "slot0": prep["slot_arrs"][0][c],
                "slot1": prep["slot_arrs"][1][c],
                "dinv_out": prep["dinv_out"][c],
            }
        )

    trace = bool(os.environ.get("K_TRACE"))
    res = run_bass_kernel_spmd(
        nc, in_maps, core_ids=list(range(NCORES)), trace=trace
    )
    global LAST_RESULT
    LAST_RESULT = res
    if trace and res.exec_time_ns is not None:
        print(f"HW exec time: {res.exec_time_ns} ns")
    mu = np.concatenate([res.results[c]["out_mu"] for c in range(NCORES)], axis=0)
    ls = np.concatenate([res.results[c]["out_ls"] for c in range(NCORES)], axis=0)
    return (mu, ls)


# revision 11
# speedup vs baseline: 1.7077x; 1.1289x over previous
"""GCNConv-pair (mu/logstd) message-passing kernel for 8 trn2 NeuronCores.

Strategy:
  - Host: fold sym-norm dinv into x rows; partition edges by destination
    core; bucket into 128-dst windows; split lo/hi by (padded) source row
    so gather indices fit int16; pad per (window, view) to the max count
    across cores so the SPMD program is uniform.
  - Device (SPMD, identical program, per-core data):
      Phase 1: hs_slice = (dinv*x_slice) @ [W_mu | W_logstd] (bf16 matmul
               over this core's 6250 nodes), AllGather -> full hs table.
      Phase 2: per (window, view) dma_gather of hs rows (4 SWDGE queues in
               parallel), one-hot chunk matmuls (TensorE) accumulate the
               segment sum per 128-dst window in PSUM, fused epilogue
               out = psum * dinv_dst + bias, DMA to outputs.
"""

import os
import sys

sys.path.insert(0, "/opt/trn_rl_repo")

import numpy as np
import ml_dtypes

import concourse.bass as bass
import concourse.bacc as bacc
import concourse.tile as tile
from concourse import mybir
from concourse.bass_utils import run_bass_kernel_spmd

# ---- problem constants (hardcoded per harness contract) ----
N_NODES = 50000
N_EDGES = 800000
IN_SIZE = 512
OUT_SIZE = 128
DOUT = 2 * OUT_SIZE  # 256, mu|logstd concatenated
NCORES = 8
NPC = N_NODES // NCORES  # 6250 nodes per core
NW = (NPC + 127) // 128  # 49 windows of 128 dst nodes
LASTW_ROWS = NPC - (NW - 1) * 128  # 106
NPC_PAD = NW * 128  # 6272 padded rows per core in the hs table
NPAD = NPC_PAD * NCORES  # 50176
SPLIT = 25088  # lo/hi padded-row split for int16 gather indices
G_WIN = 4  # windows per one-hot build group
NQ = 4  # SWDGE queues used round-robin for gathers

BF16 = ml_dtypes.bfloat16
LAST_RESULT = None


def _install_ntff_shim():
    """Register the axon NTFF profile hook if the glue module is missing."""
    try:
        import contextlib
        import ctypes
        import types

        import antenv  # noqa: F401

        if "antenv.axon_hooks" in sys.modules:
            return
        so_path = "/opt/axon/libaxon_pjrt.so"
        try:
            lib = ctypes.CDLL(so_path)
        except OSError:
            return
        if not hasattr(lib, "axon_start_nrt_profile"):
            return
        lib.axon_start_nrt_profile.argtypes = [
            ctypes.POINTER(ctypes.c_int64),
            ctypes.c_size_t,
        ]
        lib.axon_start_nrt_profile.restype = ctypes.c_int64
        lib.axon_stop_nrt_profile.argtypes = [ctypes.c_char_p]
        lib.axon_stop_nrt_profile.restype = ctypes.c_int64

        @contextlib.contextmanager
        def _hook(output_dir, device_ids):
            import jax

            jax.devices()
            if device_ids:
                ids = (ctypes.c_int64 * len(device_ids))(*device_ids)
                rc = lib.axon_start_nrt_profile(ids, len(device_ids))
            else:
                rc = lib.axon_start_nrt_profile(None, 0)
            if rc != 0:
                raise RuntimeError(f"axon_start_nrt_profile rc={rc}")
            try:
                yield
            finally:
                n = lib.axon_stop_nrt_profile(str(output_dir).encode())
                if n < 0:
                    raise RuntimeError(f"axon_stop_nrt_profile rc={n}")

        hook = _hook
        mod = types.ModuleType("antenv.axon_hooks")
        mod.set_axon_ntff_profile_hook = lambda h: None
        mod.get_axon_ntff_profile_hook = lambda: hook
        sys.modules["antenv.axon_hooks"] = mod
        antenv.axon_hooks = mod
    except Exception:
        pass


def _prep_edges(edge_index):
    """Host-side edge partitioning. Returns per-core gather structures."""
    src = np.asarray(edge_index[0], dtype=np.int64)
    dst = np.asarray(edge_index[1], dtype=np.int64)
    loops = np.arange(N_NODES, dtype=np.int64)
    src = np.concatenate([src, loops])
    dst = np.concatenate([dst, loops])

    deg = np.bincount(dst, minlength=N_NODES).astype(np.float64)
    dinv = (1.0 / np.sqrt(np.maximum(deg, 1.0))).astype(np.float32)

    core = dst // NPC
    dstl = dst - core * NPC
    win = dstl >> 7
    slot = (dstl & 127).astype(np.float32)
    # padded hs-table row of the source node
    srow = src + 22 * (src // NPC)
    view = (srow >= SPLIT).astype(np.int64)
    idx16 = np.where(view == 0, srow, srow - SPLIT)

    flat_key = (core * 2 + view) * NW + win
    cnt = np.bincount(flat_key, minlength=NCORES * 2 * NW)
    counts = cnt.reshape(NCORES, 2, NW)

    # per (view, window) gather length: max count across cores
    rwv = counts.max(axis=0)  # [2, NW]
    kwv = (rwv + 127) // 128  # chunks per (view, window)
    # chunk offsets per view (for slot arrays / matmul indexing)
    chunk_off = np.zeros((2, NW + 1), np.int64)
    chunk_off[:, 1:] = np.cumsum(kwv, axis=1)
    n_chunks = chunk_off[:, -1]  # per view
    # idx column offsets per view: full chunks (128 idx = 8 cols of 16)
    col_off = chunk_off * 8
    n_cols = col_off[:, -1]

    idx_arrs = [
        np.zeros((NCORES, 128, int(n_cols[v])), np.int16) for v in range(2)
    ]
    slot_arrs = [
        np.full((NCORES, 128, int(n_chunks[v])), -1.0, np.float32)
        for v in range(2)
    ]

    order = np.lexsort((win, view, core))
    s_src = idx16[order]
    s_slot = slot[order]
    s_key = flat_key[order]
    bucket_start = np.zeros(NCORES * 2 * NW + 1, np.int64)
    bucket_start[1:] = np.cumsum(cnt)
    rank = np.arange(src.size) - bucket_start[s_key]
    s_core = s_key // (2 * NW)
    s_view = (s_key // NW) % 2
    s_win = s_key % NW

    for v in range(2):
        S_idx = int(n_cols[v]) * 16
        S_slot = int(n_chunks[v]) * 128
        idxpos = chunk_off[v][s_win] * 128 + rank
        slotpos = idxpos
        for c in range(NCORES):
            m = (s_core == c) & (s_view == v)
            idx_flat = np.zeros(S_idx, np.int64)
            idx_flat[idxpos[m]] = s_src[m]
            # wrapped: idx i of a window lives at [i%16, coloff + i//16];
            # since window offsets are 16-aligned this is a global reshape
            w16 = idx_flat.reshape(-1, 16).T.astype(np.int16)  # [16, S/16]
            idx_arrs[v][c] = np.tile(w16, (8, 1))
            slot_flat = np.full(S_slot, -1.0, np.float32)
            slot_flat[slotpos[m]] = s_slot[m]
            slot_arrs[v][c] = slot_flat.reshape(-1, 128).T

    dinv_out = np.zeros((NCORES, 128, NW), np.float32)
    for c in range(NCORES):
        d = np.zeros(NW * 128, np.float32)
        d[:NPC] = dinv[c * NPC : (c + 1) * NPC]
        dinv_out[c] = d.reshape(NW, 128).T

    return {
        "dinv": dinv,
        "rwv": rwv,
        "kwv": kwv,
        "chunk_off": chunk_off,
        "col_off": col_off,
        "n_chunks": n_chunks,
        "n_cols": n_cols,
        "idx_arrs": [a.astype(np.int16) for a in idx_arrs],
        "slot_arrs": [a.astype(BF16) for a in slot_arrs],
        "dinv_out": dinv_out,
    }


def _build_program(prep):
    """Build the SPMD bass program (identical across cores)."""
    rwv = prep["rwv"]
    kwv = prep["kwv"]
    chunk_off = prep["chunk_off"]
    col_off = prep["col_off"]
    n_chunks = prep["n_chunks"]
    n_cols = prep["n_cols"]

    nc = bacc.Bacc(
        "TRN2",
        target_bir_lowering=False,
        debug=False,
        num_devices=NCORES,
        num_swdge_queues=NQ,
    )
    bf16 = mybir.dt.bfloat16
    f32 = mybir.dt.float32
    i16 = mybir.dt.int16

    t_xsT = nc.dram_tensor("xsT", [4, 128, NPC_PAD], bf16, kind="ExternalInput")
    t_w = nc.dram_tensor("wcat", [4, 128, DOUT], bf16, kind="ExternalInput")
    t_bias = nc.dram_tensor("bias", [128, DOUT], f32, kind="ExternalInput")
    t_iota = nc.dram_tensor("iota", [128, 128], bf16, kind="ExternalInput")
    t_idx0 = nc.dram_tensor("idx0", [128, int(n_cols[0])], i16, kind="ExternalInput")
    t_idx1 = nc.dram_tensor("idx1", [128, int(n_cols[1])], i16, kind="ExternalInput")
    t_slot0 = nc.dram_tensor("slot0", [128, int(n_chunks[0])], bf16, kind="ExternalInput")
    t_slot1 = nc.dram_tensor("slot1", [128, int(n_chunks[1])], bf16, kind="ExternalInput")
    t_dinv = nc.dram_tensor("dinv_out", [128, NW], f32, kind="ExternalInput")
    t_mu = nc.dram_tensor("out_mu", [NPC, OUT_SIZE], f32, kind="ExternalOutput")
    t_ls = nc.dram_tensor("out_ls", [NPC, OUT_SIZE], f32, kind="ExternalOutput")
    t_idx = [t_idx0, t_idx1]
    t_slot = [t_slot0, t_slot1]

    with tile.TileContext(nc) as tc:
        with (
            tc.tile_pool(name="dram", bufs=1, space="DRAM") as dram,
            tc.tile_pool(name="const", bufs=1) as const,
            tc.tile_pool(name="hst", bufs=4) as hstp,
            tc.tile_pool(name="msg", bufs=2) as msgp,
            tc.tile_pool(name="oh", bufs=2) as ohp,
            tc.tile_pool(name="res", bufs=4) as resp,
            tc.tile_pool(name="psum", bufs=4, space="PSUM") as psp,
        ):
            hs_bounce = dram.tile([NPC_PAD, DOUT], bf16)
            hs_full = dram.tile([NPAD, DOUT], bf16, name="hs_full", addr_space="Shared")

            # constants (live for the whole kernel)
            w_tiles = []
            for kt in range(4):
                wt = const.tile([128, DOUT], bf16, name=f"w{kt}")
                nc.sync.dma_start(wt[:], t_w[kt])
                w_tiles.append(wt)
            bias_t = const.tile([128, DOUT], f32, name="bias_t")
            nc.sync.dma_start(bias_t[:], t_bias[:])
            iota_t = const.tile([128, 128], bf16, name="iota_t")
            nc.sync.dma_start(iota_t[:], t_iota[:])
            dinv_t = const.tile([128, NW], f32, name="dinv_t")
            nc.sync.dma_start(dinv_t[:], t_dinv[:])
            # full idx / slot tables resident in SBUF
            idx_c = []
            slot_c = []
            for v in range(2):
                it = const.tile([128, int(n_cols[v])], i16, name=f"idxc{v}")
                nc.sync.dma_start(it[:], t_idx[v][:])
                idx_c.append(it)
                st = const.tile([128, int(n_chunks[v])], bf16, name=f"slotc{v}")
                nc.sync.dma_start(st[:], t_slot[v][:])
                slot_c.append(st)

            # ---- Phase 1: hs_slice = xs @ Wcat, then AllGather ----
            with tc.tile_pool(name="xkp", bufs=1) as xkp:
                xk = []
                for kt in range(4):
                    xt_ = xkp.tile([128, NPC_PAD], bf16, name=f"xk{kt}")
                    nc.sync.dma_start(xt_[:], t_xsT[kt])
                    xk.append(xt_)
                for g in range(0, NW, 4):
                    gn = min(4, NW - g)
                    hstage = hstp.tile([128, 4, DOUT], bf16, name="hstage")
                    for j in range(gn):
                        nb = g + j
                        ph = psp.tile([128, DOUT], f32, name="ph", tag="ph")
                        for kt in range(4):
                            nc.tensor.matmul(
                                out=ph[:],
                                lhsT=xk[kt][:, nb * 128 : (nb + 1) * 128],
                                rhs=w_tiles[kt][:],
                                start=(kt == 0),
                                stop=(kt == 3),
                            )
                        nc.vector.tensor_copy(hstage[:, j, :], ph[:])
                    nc.sync.dma_start(
                        hs_bounce[:]
                        .rearrange("(a p) d -> p a d", p=128)[:, g : g + gn, :],
                        hstage[:, :gn, :],
                    )
                # hs_bounce rows are node-major: row = a*128 + p
                nc.gpsimd.collective_compute(
                    "AllGather",
                    mybir.AluOpType.bypass,
                    replica_groups=[list(range(NCORES))],
                    ins=[hs_bounce[:].opt()],
                    outs=[hs_full[:].opt()],
                )

            hs_ap = hs_full[:]
            views = [hs_ap[0 : SPLIT + 128, :], hs_ap[SPLIT:NPAD, :]]

            # ---- Phase 2: grouped gathers + one-hot segment-sum ----
            chmax = [
                max(
                    int(chunk_off[v][min(a + G_WIN, NW)] - chunk_off[v][a])
                    for a in range(0, NW, G_WIN)
                )
                for v in range(2)
            ]
            qn = 0
            for g0 in range(0, NW, G_WIN):
                g1 = min(g0 + G_WIN, NW)
                oh = [None, None]
                msgs = [None, None]
                for v in range(2):
                    ch0, ch1 = int(chunk_off[v][g0]), int(chunk_off[v][g1])
                    ch = ch1 - ch0
                    oh[v] = ohp.tile(
                        [128, chmax[v], 128], bf16, name=f"oh{v}", tag=f"oh{v}"
                    )
                    nc.vector.tensor_tensor(
                        out=oh[v][:, :ch, :],
                        in0=slot_c[v][:, ch0:ch1]
                        .unsqueeze(-1)
                        .broadcast_to([128, ch, 128]),
                        in1=iota_t[:].unsqueeze(1).broadcast_to([128, ch, 128]),
                        op=mybir.AluOpType.is_equal,
                    )
                    mt = msgp.tile(
                        [128, chmax[v], DOUT], bf16, name=f"msg{v}", tag=f"msg{v}"
                    )
                    nc.gpsimd.dma_gather(
                        out_ap=mt[:, :ch, :],
                        in_ap=views[v],
                        idxs_ap=idx_c[v][:, ch0 * 8 : ch1 * 8],
                        num_idxs=ch * 128,
                        num_idxs_reg=ch * 128,
                        elem_size=DOUT,
                        single_packet=False,
                        queue_num=qn,
                    )
                    qn = (qn + 1) % NQ
                    msgs[v] = mt
                for w in range(g0, g1):
                    po = psp.tile([128, DOUT], f32, name="po", tag="po")
                    nmm = int(kwv[0][w] + kwv[1][w])
                    i = 0
                    for v in range(2):
                        kbase = int(chunk_off[v][w]) - int(chunk_off[v][g0])
                        for k in range(int(kwv[v][w])):
                            nc.tensor.matmul(
                                out=po[:],
                                lhsT=oh[v][:, kbase + k, :],
                                rhs=msgs[v][:, kbase + k, :],
                                start=(i == 0),
                                stop=(i == nmm - 1),
                            )
                            i += 1
                    res = resp.tile([128, DOUT], f32, name="res", tag="res")
                    nc.vector.scalar_tensor_tensor(
                        out=res[:],
                        in0=po[:],
                        scalar=dinv_t[:, w : w + 1],
                        in1=bias_t[:],
                        op0=mybir.AluOpType.mult,
                        op1=mybir.AluOpType.add,
                    )
                    rows = LASTW_ROWS if w == NW - 1 else 128
                    r0 = w * 128
                    nc.sync.dma_start(t_mu[r0 : r0 + rows, :], res[:rows, 0:OUT_SIZE])
                    nc.sync.dma_start(
                        t_ls[r0 : r0 + rows, :], res[:rows, OUT_SIZE:DOUT]
                    )

    nc.compile()
    return nc


def kernel(x, edge_index, W_mu, b_mu, W_logstd, b_logstd):
    _install_ntff_shim()

    x = np.asarray(x, dtype=np.float32)
    prep = _prep_edges(np.asarray(edge_index))

    # fold dinv into x rows; per-core transposed slice [4,128,NPC_PAD]
    xs = (x * prep["dinv"][:, None]).astype(BF16)
    xsT_cores = []
    for c in range(NCORES):
        sl = np.zeros((IN_SIZE, NPC_PAD), BF16)
        sl[:, :NPC] = xs[c * NPC : (c + 1) * NPC].T
        xsT_cores.append(np.ascontiguousarray(sl.reshape(4, 128, NPC_PAD)))

    wcat = np.concatenate(
        [np.asarray(W_mu, np.float32), np.asarray(W_logstd, np.float32)], axis=1
    ).astype(BF16)
    wcat_t = np.ascontiguousarray(wcat.reshape(4, 128, DOUT))
    bias = np.concatenate(
        [np.asarray(b_mu, np.float32), np.asarray(b_logstd, np.float32)]
    ).astype(np.float32)
    bias_rep = np.tile(bias[None, :], (128, 1))
    iota_arr = np.tile(
        np.arange(128, dtype=np.float32).astype(BF16)[None, :], (128, 1)
    )

    nc = _build_program(prep)

    in_maps = []
    for c in range(NCORES):
        in_maps.append(
            {
                "xsT": xsT_cores[c],
                "wcat": wcat_t,
                "bias": bias_rep,
                "iota": iota_arr,
                "idx0": prep["idx_arrs"][0][c],
                "idx1": prep["idx_arrs"][1][c],
                "slot0": prep["slot_arrs"][0][c],
                "slot1": prep["slot_arrs"][1][c],
                "dinv_out": prep["dinv_out"][c],
            }
        )

    trace = bool(os.environ.get("K_TRACE"))
    res = run_bass_kernel_spmd(
        nc, in_maps, core_ids=list(range(NCORES)), trace=trace
    )
    global LAST_RESULT
    LAST_RESULT = res
    if trace and res.exec_time_ns is not None:
        print(f"HW exec time: {res.exec_time_ns} ns")
    mu = np.concatenate([res.results[c]["out_mu"] for c in range(NCORES)], axis=0)
    ls = np.concatenate([res.results[c]["out_ls"] for c in range(NCORES)], axis=0)
    return (mu, ls)


# revision 12
# speedup vs baseline: 1.8240x; 1.0681x over previous
"""GCNConv-pair (mu/logstd) message-passing kernel for 8 trn2 NeuronCores.

Strategy:
  - Host: fold sym-norm dinv into x rows; partition edges by destination
    core; bucket into 128-dst windows; split lo/hi by (padded) source row
    so gather indices fit int16; pad per (window, view) to the max count
    across cores so the SPMD program is uniform.
  - Device (SPMD, identical program, per-core data):
      Phase 1: hs_slice = (dinv*x_slice) @ [W_mu | W_logstd] (bf16 matmul
               over this core's 6250 nodes), AllGather -> full hs table.
      Phase 2: per (window, view) dma_gather of hs rows (4 SWDGE queues in
               parallel), one-hot chunk matmuls (TensorE) accumulate the
               segment sum per 128-dst window in PSUM, fused epilogue
               out = psum * dinv_dst + bias, DMA to outputs.
"""

import os
import sys

sys.path.insert(0, "/opt/trn_rl_repo")

import numpy as np
import ml_dtypes

import concourse.bass as bass
import concourse.bacc as bacc
import concourse.tile as tile
from concourse import mybir
from concourse.bass_utils import run_bass_kernel_spmd

# ---- problem constants (hardcoded per harness contract) ----
N_NODES = 50000
N_EDGES = 800000
IN_SIZE = 512
OUT_SIZE = 128
DOUT = 2 * OUT_SIZE  # 256, mu|logstd concatenated
NCORES = 8
NPC = N_NODES // NCORES  # 6250 nodes per core
NW = (NPC + 127) // 128  # 49 windows of 128 dst nodes
LASTW_ROWS = NPC - (NW - 1) * 128  # 106
NPC_PAD = NW * 128  # 6272 padded rows per core in the hs table
NPAD = NPC_PAD * NCORES  # 50176
SPLIT = 25088  # lo/hi padded-row split for int16 gather indices
G_WIN = 2  # windows per one-hot build group
NQ = 4  # SWDGE queues used round-robin for gathers

BF16 = ml_dtypes.bfloat16
LAST_RESULT = None


def _install_ntff_shim():
    """Register the axon NTFF profile hook if the glue module is missing."""
    try:
        import contextlib
        import ctypes
        import types

        import antenv  # noqa: F401

        if "antenv.axon_hooks" in sys.modules:
            return
        so_path = "/opt/axon/libaxon_pjrt.so"
        try:
            lib = ctypes.CDLL(so_path)
        except OSError:
            return
        if not hasattr(lib, "axon_start_nrt_profile"):
            return
        lib.axon_start_nrt_profile.argtypes = [
            ctypes.POINTER(ctypes.c_int64),
            ctypes.c_size_t,
        ]
        lib.axon_start_nrt_profile.restype = ctypes.c_int64
        lib.axon_stop_nrt_profile.argtypes = [ctypes.c_char_p]
        lib.axon_stop_nrt_profile.restype = ctypes.c_int64

        @contextlib.contextmanager
        def _hook(output_dir, device_ids):
            import jax

            jax.devices()
            if device_ids:
                ids = (ctypes.c_int64 * len(device_ids))(*device_ids)
                rc = lib.axon_start_nrt_profile(ids, len(device_ids))
            else:
                rc = lib.axon_start_nrt_profile(None, 0)
            if rc != 0:
                raise RuntimeError(f"axon_start_nrt_profile rc={rc}")
            try:
                yield
            finally:
                n = lib.axon_stop_nrt_profile(str(output_dir).encode())
                if n < 0:
                    raise RuntimeError(f"axon_stop_nrt_profile rc={n}")

        hook = _hook
        mod = types.ModuleType("antenv.axon_hooks")
        mod.set_axon_ntff_profile_hook = lambda h: None
        mod.get_axon_ntff_profile_hook = lambda: hook
        sys.modules["antenv.axon_hooks"] = mod
        antenv.axon_hooks = mod
    except Exception:
        pass


def _prep_edges(edge_index):
    """Host-side edge partitioning. Returns per-core gather structures."""
    src = np.asarray(edge_index[0], dtype=np.int64)
    dst = np.asarray(edge_index[1], dtype=np.int64)
    loops = np.arange(N_NODES, dtype=np.int64)
    src = np.concatenate([src, loops])
    dst = np.concatenate([dst, loops])

    deg = np.bincount(dst, minlength=N_NODES).astype(np.float64)
    dinv = (1.0 / np.sqrt(np.maximum(deg, 1.0))).astype(np.float32)

    core = dst // NPC
    dstl = dst - core * NPC
    win = dstl >> 7
    slot = (dstl & 127).astype(np.float32)
    # padded hs-table row of the source node
    srow = src + 22 * (src // NPC)
    view = (srow >= SPLIT).astype(np.int64)
    idx16 = np.where(view == 0, srow, srow - SPLIT)

    flat_key = (core * 2 + view) * NW + win
    cnt = np.bincount(flat_key, minlength=NCORES * 2 * NW)
    counts = cnt.reshape(NCORES, 2, NW)

    # per (view, window) gather length: max count across cores
    rwv = counts.max(axis=0)  # [2, NW]
    kwv = (rwv + 127) // 128  # chunks per (view, window)
    # chunk offsets per view (for slot arrays / matmul indexing)
    chunk_off = np.zeros((2, NW + 1), np.int64)
    chunk_off[:, 1:] = np.cumsum(kwv, axis=1)
    n_chunks = chunk_off[:, -1]  # per view
    # idx column offsets per view: full chunks (128 idx = 8 cols of 16)
    col_off = chunk_off * 8
    n_cols = col_off[:, -1]

    idx_arrs = [
        np.zeros((NCORES, 128, int(n_cols[v])), np.int16) for v in range(2)
    ]
    slot_arrs = [
        np.full((NCORES, 128, int(n_chunks[v])), -1.0, np.float32)
        for v in range(2)
    ]

    order = np.lexsort((win, view, core))
    s_src = idx16[order]
    s_slot = slot[order]
    s_key = flat_key[order]
    bucket_start = np.zeros(NCORES * 2 * NW + 1, np.int64)
    bucket_start[1:] = np.cumsum(cnt)
    rank = np.arange(src.size) - bucket_start[s_key]
    s_core = s_key // (2 * NW)
    s_view = (s_key // NW) % 2
    s_win = s_key % NW

    for v in range(2):
        S_idx = int(n_cols[v]) * 16
        S_slot = int(n_chunks[v]) * 128
        idxpos = chunk_off[v][s_win] * 128 + rank
        slotpos = idxpos
        for c in range(NCORES):
            m = (s_core == c) & (s_view == v)
            idx_flat = np.zeros(S_idx, np.int64)
            idx_flat[idxpos[m]] = s_src[m]
            # wrapped: idx i of a window lives at [i%16, coloff + i//16];
            # since window offsets are 16-aligned this is a global reshape
            w16 = idx_flat.reshape(-1, 16).T.astype(np.int16)  # [16, S/16]
            idx_arrs[v][c] = np.tile(w16, (8, 1))
            slot_flat = np.full(S_slot, -1.0, np.float32)
            slot_flat[slotpos[m]] = s_slot[m]
            slot_arrs[v][c] = slot_flat.reshape(-1, 128).T

    dinv_out = np.zeros((NCORES, 128, NW), np.float32)
    for c in range(NCORES):
        d = np.zeros(NW * 128, np.float32)
        d[:NPC] = dinv[c * NPC : (c + 1) * NPC]
        dinv_out[c] = d.reshape(NW, 128).T

    return {
        "dinv": dinv,
        "rwv": rwv,
        "kwv": kwv,
        "chunk_off": chunk_off,
        "col_off": col_off,
        "n_chunks": n_chunks,
        "n_cols": n_cols,
        "idx_arrs": [a.astype(np.int16) for a in idx_arrs],
        "slot_arrs": [a.astype(BF16) for a in slot_arrs],
        "dinv_out": dinv_out,
    }


def _build_program(prep):
    """Build the SPMD bass program (identical across cores)."""
    rwv = prep["rwv"]
    kwv = prep["kwv"]
    chunk_off = prep["chunk_off"]
    col_off = prep["col_off"]
    n_chunks = prep["n_chunks"]
    n_cols = prep["n_cols"]

    nc = bacc.Bacc(
        "TRN2",
        target_bir_lowering=False,
        debug=False,
        num_devices=NCORES,
        num_swdge_queues=NQ,
    )
    bf16 = mybir.dt.bfloat16
    f32 = mybir.dt.float32
    i16 = mybir.dt.int16

    t_xsT = nc.dram_tensor("xsT", [4, 128, NPC_PAD], bf16, kind="ExternalInput")
    t_w = nc.dram_tensor("wcat", [4, 128, DOUT], bf16, kind="ExternalInput")
    t_bias = nc.dram_tensor("bias", [128, DOUT], f32, kind="ExternalInput")
    t_iota = nc.dram_tensor("iota", [128, 128], bf16, kind="ExternalInput")
    t_idx0 = nc.dram_tensor("idx0", [128, int(n_cols[0])], i16, kind="ExternalInput")
    t_idx1 = nc.dram_tensor("idx1", [128, int(n_cols[1])], i16, kind="ExternalInput")
    t_slot0 = nc.dram_tensor("slot0", [128, int(n_chunks[0])], bf16, kind="ExternalInput")
    t_slot1 = nc.dram_tensor("slot1", [128, int(n_chunks[1])], bf16, kind="ExternalInput")
    t_dinv = nc.dram_tensor("dinv_out", [128, NW], f32, kind="ExternalInput")
    t_mu = nc.dram_tensor("out_mu", [NPC, OUT_SIZE], f32, kind="ExternalOutput")
    t_ls = nc.dram_tensor("out_ls", [NPC, OUT_SIZE], f32, kind="ExternalOutput")
    t_idx = [t_idx0, t_idx1]
    t_slot = [t_slot0, t_slot1]

    with tile.TileContext(nc) as tc:
        with (
            tc.tile_pool(name="dram", bufs=1, space="DRAM") as dram,
            tc.tile_pool(name="const", bufs=1) as const,
            tc.tile_pool(name="hst", bufs=4) as hstp,
            tc.tile_pool(name="msg", bufs=4) as msgp,
            tc.tile_pool(name="idxp", bufs=4) as idxp,
            tc.tile_pool(name="oh", bufs=3) as ohp,
            tc.tile_pool(name="res", bufs=4) as resp,
            tc.tile_pool(name="psum", bufs=4, space="PSUM") as psp,
        ):
            hs_bounce = dram.tile([NPC_PAD, DOUT], bf16)
            hs_full = dram.tile([NPAD, DOUT], bf16, name="hs_full", addr_space="Shared")

            # constants (live for the whole kernel)
            w_tiles = []
            for kt in range(4):
                wt = const.tile([128, DOUT], bf16, name=f"w{kt}")
                nc.sync.dma_start(wt[:], t_w[kt])
                w_tiles.append(wt)
            bias_t = const.tile([128, DOUT], f32, name="bias_t")
            nc.sync.dma_start(bias_t[:], t_bias[:])
            iota_t = const.tile([128, 128], bf16, name="iota_t")
            nc.sync.dma_start(iota_t[:], t_iota[:])
            dinv_t = const.tile([128, NW], f32, name="dinv_t")
            nc.sync.dma_start(dinv_t[:], t_dinv[:])
            # full slot tables resident in SBUF (DVE reads strided slices)
            slot_c = []
            for v in range(2):
                st = const.tile([128, int(n_chunks[v])], bf16, name=f"slotc{v}")
                nc.sync.dma_start(st[:], t_slot[v][:])
                slot_c.append(st)

            # ---- Phase 1: hs_slice = xs @ Wcat, then AllGather ----
            with tc.tile_pool(name="xkp", bufs=1) as xkp:
                xk = []
                for kt in range(4):
                    xt_ = xkp.tile([128, NPC_PAD], bf16, name=f"xk{kt}")
                    nc.sync.dma_start(xt_[:], t_xsT[kt])
                    xk.append(xt_)
                for g in range(0, NW, 4):
                    gn = min(4, NW - g)
                    hstage = hstp.tile([128, 4, DOUT], bf16, name="hstage")
                    for j in range(gn):
                        nb = g + j
                        ph = psp.tile([128, DOUT], f32, name="ph", tag="ph")
                        for kt in range(4):
                            nc.tensor.matmul(
                                out=ph[:],
                                lhsT=xk[kt][:, nb * 128 : (nb + 1) * 128],
                                rhs=w_tiles[kt][:],
                                start=(kt == 0),
                                stop=(kt == 3),
                            )
                        nc.vector.tensor_copy(hstage[:, j, :], ph[:])
                    nc.sync.dma_start(
                        hs_bounce[:]
                        .rearrange("(a p) d -> p a d", p=128)[:, g : g + gn, :],
                        hstage[:, :gn, :],
                    )
                # hs_bounce rows are node-major: row = a*128 + p
                nc.gpsimd.collective_compute(
                    "AllGather",
                    mybir.AluOpType.bypass,
                    replica_groups=[list(range(NCORES))],
                    ins=[hs_bounce[:].opt()],
                    outs=[hs_full[:].opt()],
                )

            hs_ap = hs_full[:]
            views = [hs_ap[0 : SPLIT + 128, :], hs_ap[SPLIT:NPAD, :]]

            # ---- Phase 2: grouped gathers + one-hot segment-sum ----
            chmax = [
                max(
                    int(chunk_off[v][min(a + G_WIN, NW)] - chunk_off[v][a])
                    for a in range(0, NW, G_WIN)
                )
                for v in range(2)
            ]
            qn = 0
            for g0 in range(0, NW, G_WIN):
                g1 = min(g0 + G_WIN, NW)
                oh = [None, None]
                msgs = [None, None]
                for v in range(2):
                    ch0, ch1 = int(chunk_off[v][g0]), int(chunk_off[v][g1])
                    ch = ch1 - ch0
                    oh[v] = ohp.tile(
                        [128, chmax[v], 128], bf16, name=f"oh{v}", tag=f"oh{v}"
                    )
                    nc.vector.tensor_tensor(
                        out=oh[v][:, :ch, :],
                        in0=slot_c[v][:, ch0:ch1]
                        .unsqueeze(-1)
                        .broadcast_to([128, ch, 128]),
                        in1=iota_t[:].unsqueeze(1).broadcast_to([128, ch, 128]),
                        op=mybir.AluOpType.is_equal,
                    )
                    idx_t = idxp.tile(
                        [128, chmax[v] * 8], i16, name=f"idx{v}", tag=f"idx{v}"
                    )
                    nc.sync.dma_start(idx_t[:, : ch * 8], t_idx[v][:, ch0 * 8 : ch1 * 8])
                    mt = msgp.tile(
                        [128, chmax[v], DOUT], bf16, name=f"msg{v}", tag=f"msg{v}"
                    )
                    nc.gpsimd.dma_gather(
                        out_ap=mt[:, :ch, :],
                        in_ap=views[v],
                        idxs_ap=idx_t[:, : ch * 8],
                        num_idxs=ch * 128,
                        num_idxs_reg=ch * 128,
                        elem_size=DOUT,
                        single_packet=False,
                        queue_num=qn,
                    )
                    qn = (qn + 1) % NQ
                    msgs[v] = mt
                for w in range(g0, g1):
                    po = psp.tile([128, DOUT], f32, name="po", tag="po")
                    nmm = int(kwv[0][w] + kwv[1][w])
                    i = 0
                    for v in range(2):
                        kbase = int(chunk_off[v][w]) - int(chunk_off[v][g0])
                        for k in range(int(kwv[v][w])):
                            nc.tensor.matmul(
                                out=po[:],
                                lhsT=oh[v][:, kbase + k, :],
                                rhs=msgs[v][:, kbase + k, :],
                                start=(i == 0),
                                stop=(i == nmm - 1),
                            )
                            i += 1
                    res = resp.tile([128, DOUT], f32, name="res", tag="res")
                    nc.vector.scalar_tensor_tensor(
                        out=res[:],
                        in0=po[:],
                        scalar=dinv_t[:, w : w + 1],
                        in1=bias_t[:],
                        op0=mybir.AluOpType.mult,
                        op1=mybir.AluOpType.add,
                    )
                    rows = LASTW_ROWS if w == NW - 1 else 128
                    r0 = w * 128
                    nc.sync.dma_start(t_mu[r0 : r0 + rows, :], res[:rows, 0:OUT_SIZE])
                    nc.sync.dma_start(
                        t_ls[r0 : r0 + rows, :], res[:rows, OUT_SIZE:DOUT]
                    )

    nc.compile()
    return nc


def kernel(x, edge_index, W_mu, b_mu, W_logstd, b_logstd):
    _install_ntff_shim()

    x = np.asarray(x, dtype=np.float32)
    prep = _prep_edges(np.asarray(edge_index))

    # fold dinv into x rows; per-core transposed slice [4,128,NPC_PAD]
    xs = (x * prep["dinv"][:, None]).astype(BF16)
    xsT_cores = []
    for c in range(NCORES):
        sl = np.zeros((IN_SIZE, NPC_PAD), BF16)
        sl[:, :NPC] = xs[c * NPC : (c + 1) * NPC].T
        xsT_cores.append(np.ascontiguousarray(sl.reshape(4, 128, NPC_PAD)))

    wcat = np.concatenate(
        [np.asarray(W_mu, np.float32), np.asarray(W_logstd, np.float32)], axis=1
    ).astype(BF16)
    wcat_t = np.ascontiguousarray(wcat.reshape(4, 128, DOUT))
    bias = np.concatenate(
        [np.asarray(b_mu, np.float32), np.asarray(b_logstd, np.float32)]
    ).astype(np.float32)
    bias_rep = np.tile(bias[None, :], (128, 1))
    iota_arr = np.tile(
        np.arange(128, dtype=np.float32).astype(BF16)[None, :], (128, 1)
    )

    nc = _build_program(prep)

    in_maps = []
    for c in range(NCORES):
        in_maps.append(
            {
                "xsT": xsT_cores[c],
                "wcat": wcat_t,
                "bias": bias_rep,
                "iota": iota_arr,
                "idx0": prep["idx_arrs"][0][c],
                "idx1": prep["idx_arrs"][1][c],
                "slot0": prep["slot_arrs"][0][c],
                "slot1": prep["slot_arrs"][1][c],
                "dinv_out": prep["dinv_out"][c],
            }
        )

    trace = bool(os.environ.get("K_TRACE"))
    res = run_bass_kernel_spmd(
        nc, in_maps, core_ids=list(range(NCORES)), trace=trace
    )
    global LAST_RESULT
    LAST_RESULT = res
    if trace and res.exec_time_ns is not None:
        print(f"HW exec time: {res.exec_time_ns} ns")
    mu = np.concatenate([res.results[c]["out_mu"] for c in range(NCORES)], axis=0)
    ls = np.concatenate([res.results[c]["out_ls"] for c in range(NCORES)], axis=0)
    return (mu, ls)


# revision 13
# speedup vs baseline: 1.8470x; 1.0126x over previous
"""GCNConv-pair (mu/logstd) message-passing kernel for 8 trn2 NeuronCores.

Strategy:
  - Host: fold sym-norm dinv into x rows; partition edges by destination
    core; bucket into 128-dst windows; split lo/hi by (padded) source row
    so gather indices fit int16; pad per (window, view) to the max count
    across cores so the SPMD program is uniform.
  - Device (SPMD, identical program, per-core data):
      Phase 1: hs_slice = (dinv*x_slice) @ [W_mu | W_logstd] (bf16 matmul
               over this core's 6250 nodes), AllGather -> full hs table.
      Phase 2: per (window, view) dma_gather of hs rows (4 SWDGE queues in
               parallel), one-hot chunk matmuls (TensorE) accumulate the
               segment sum per 128-dst window in PSUM, fused epilogue
               out = psum * dinv_dst + bias, DMA to outputs.
"""

import os
import sys

sys.path.insert(0, "/opt/trn_rl_repo")

import numpy as np
import ml_dtypes

import concourse.bass as bass
import concourse.bacc as bacc
import concourse.tile as tile
from concourse import mybir
from concourse.bass_utils import run_bass_kernel_spmd

# ---- problem constants (hardcoded per harness contract) ----
N_NODES = 50000
N_EDGES = 800000
IN_SIZE = 512
OUT_SIZE = 128
DOUT = 2 * OUT_SIZE  # 256, mu|logstd concatenated
NCORES = 8
NPC = N_NODES // NCORES  # 6250 nodes per core
NW = (NPC + 127) // 128  # 49 windows of 128 dst nodes
LASTW_ROWS = NPC - (NW - 1) * 128  # 106
NPC_PAD = NW * 128  # 6272 padded rows per core in the hs table
NPAD = NPC_PAD * NCORES  # 50176
SPLIT = 25088  # lo/hi padded-row split for int16 gather indices
G_WIN = 1  # windows per one-hot build group
NQ = 4  # SWDGE queues used round-robin for gathers

BF16 = ml_dtypes.bfloat16
LAST_RESULT = None


def _install_ntff_shim():
    """Register the axon NTFF profile hook if the glue module is missing."""
    try:
        import contextlib
        import ctypes
        import types

        import antenv  # noqa: F401

        if "antenv.axon_hooks" in sys.modules:
            return
        so_path = "/opt/axon/libaxon_pjrt.so"
        try:
            lib = ctypes.CDLL(so_path)
        except OSError:
            return
        if not hasattr(lib, "axon_start_nrt_profile"):
            return
        lib.axon_start_nrt_profile.argtypes = [
            ctypes.POINTER(ctypes.c_int64),
            ctypes.c_size_t,
        ]
        lib.axon_start_nrt_profile.restype = ctypes.c_int64
        lib.axon_stop_nrt_profile.argtypes = [ctypes.c_char_p]
        lib.axon_stop_nrt_profile.restype = ctypes.c_int64

        @contextlib.contextmanager
        def _hook(output_dir, device_ids):
            import jax

            jax.devices()
            if device_ids:
                ids = (ctypes.c_int64 * len(device_ids))(*device_ids)
                rc = lib.axon_start_nrt_profile(ids, len(device_ids))
            else:
                rc = lib.axon_start_nrt_profile(None, 0)
            if rc != 0:
                raise RuntimeError(f"axon_start_nrt_profile rc={rc}")
            try:
                yield
            finally:
                n = lib.axon_stop_nrt_profile(str(output_dir).encode())
                if n < 0:
                    raise RuntimeError(f"axon_stop_nrt_profile rc={n}")

        hook = _hook
        mod = types.ModuleType("antenv.axon_hooks")
        mod.set_axon_ntff_profile_hook = lambda h: None
        mod.get_axon_ntff_profile_hook = lambda: hook
        sys.modules["antenv.axon_hooks"] = mod
        antenv.axon_hooks = mod
    except Exception:
        pass


def _prep_edges(edge_index):
    """Host-side edge partitioning. Returns per-core gather structures."""
    src = np.asarray(edge_index[0], dtype=np.int64)
    dst = np.asarray(edge_index[1], dtype=np.int64)
    loops = np.arange(N_NODES, dtype=np.int64)
    src = np.concatenate([src, loops])
    dst = np.concatenate([dst, loops])

    deg = np.bincount(dst, minlength=N_NODES).astype(np.float64)
    dinv = (1.0 / np.sqrt(np.maximum(deg, 1.0))).astype(np.float32)

    core = dst // NPC
    dstl = dst - core * NPC
    win = dstl >> 7
    slot = (dstl & 127).astype(np.float32)
    # padded hs-table row of the source node
    srow = src + 22 * (src // NPC)
    view = (srow >= SPLIT).astype(np.int64)
    idx16 = np.where(view == 0, srow, srow - SPLIT)

    flat_key = (core * 2 + view) * NW + win
    cnt = np.bincount(flat_key, minlength=NCORES * 2 * NW)
    counts = cnt.reshape(NCORES, 2, NW)

    # per (view, window) gather length: max count across cores
    rwv = counts.max(axis=0)  # [2, NW]
    kwv = (rwv + 127) // 128  # chunks per (view, window)
    # chunk offsets per view (for slot arrays / matmul indexing)
    chunk_off = np.zeros((2, NW + 1), np.int64)
    chunk_off[:, 1:] = np.cumsum(kwv, axis=1)
    n_chunks = chunk_off[:, -1]  # per view
    # idx column offsets per view: full chunks (128 idx = 8 cols of 16)
    col_off = chunk_off * 8
    n_cols = col_off[:, -1]

    idx_arrs = [
        np.zeros((NCORES, 128, int(n_cols[v])), np.int16) for v in range(2)
    ]
    slot_arrs = [
        np.full((NCORES, 128, int(n_chunks[v])), -1.0, np.float32)
        for v in range(2)
    ]

    order = np.lexsort((win, view, core))
    s_src = idx16[order]
    s_slot = slot[order]
    s_key = flat_key[order]
    bucket_start = np.zeros(NCORES * 2 * NW + 1, np.int64)
    bucket_start[1:] = np.cumsum(cnt)
    rank = np.arange(src.size) - bucket_start[s_key]
    s_core = s_key // (2 * NW)
    s_view = (s_key // NW) % 2
    s_win = s_key % NW

    for v in range(2):
        S_idx = int(n_cols[v]) * 16
        S_slot = int(n_chunks[v]) * 128
        idxpos = chunk_off[v][s_win] * 128 + rank
        slotpos = idxpos
        for c in range(NCORES):
            m = (s_core == c) & (s_view == v)
            idx_flat = np.zeros(S_idx, np.int64)
            idx_flat[idxpos[m]] = s_src[m]
            # wrapped: idx i of a window lives at [i%16, coloff + i//16];
            # since window offsets are 16-aligned this is a global reshape
            w16 = idx_flat.reshape(-1, 16).T.astype(np.int16)  # [16, S/16]
            idx_arrs[v][c] = np.tile(w16, (8, 1))
            slot_flat = np.full(S_slot, -1.0, np.float32)
            slot_flat[slotpos[m]] = s_slot[m]
            slot_arrs[v][c] = slot_flat.reshape(-1, 128).T

    dinv_out = np.zeros((NCORES, 128, NW), np.float32)
    for c in range(NCORES):
        d = np.zeros(NW * 128, np.float32)
        d[:NPC] = dinv[c * NPC : (c + 1) * NPC]
        dinv_out[c] = d.reshape(NW, 128).T

    return {
        "dinv": dinv,
        "rwv": rwv,
        "kwv": kwv,
        "chunk_off": chunk_off,
        "col_off": col_off,
        "n_chunks": n_chunks,
        "n_cols": n_cols,
        "idx_arrs": [a.astype(np.int16) for a in idx_arrs],
        "slot_arrs": [a.astype(BF16) for a in slot_arrs],
        "dinv_out": dinv_out,
    }


def _build_program(prep):
    """Build the SPMD bass program (identical across cores)."""
    rwv = prep["rwv"]
    kwv = prep["kwv"]
    chunk_off = prep["chunk_off"]
    col_off = prep["col_off"]
    n_chunks = prep["n_chunks"]
    n_cols = prep["n_cols"]

    nc = bacc.Bacc(
        "TRN2",
        target_bir_lowering=False,
        debug=False,
        num_devices=NCORES,
        num_swdge_queues=NQ,
    )
    bf16 = mybir.dt.bfloat16
    f32 = mybir.dt.float32
    i16 = mybir.dt.int16

    t_xsT = nc.dram_tensor("xsT", [4, 128, NPC_PAD], bf16, kind="ExternalInput")
    t_w = nc.dram_tensor("wcat", [4, 128, DOUT], bf16, kind="ExternalInput")
    t_bias = nc.dram_tensor("bias", [128, DOUT], f32, kind="ExternalInput")
    t_iota = nc.dram_tensor("iota", [128, 128], bf16, kind="ExternalInput")
    t_idx0 = nc.dram_tensor("idx0", [128, int(n_cols[0])], i16, kind="ExternalInput")
    t_idx1 = nc.dram_tensor("idx1", [128, int(n_cols[1])], i16, kind="ExternalInput")
    t_slot0 = nc.dram_tensor("slot0", [128, int(n_chunks[0])], bf16, kind="ExternalInput")
    t_slot1 = nc.dram_tensor("slot1", [128, int(n_chunks[1])], bf16, kind="ExternalInput")
    t_dinv = nc.dram_tensor("dinv_out", [128, NW], f32, kind="ExternalInput")
    t_mu = nc.dram_tensor("out_mu", [NPC, OUT_SIZE], f32, kind="ExternalOutput")
    t_ls = nc.dram_tensor("out_ls", [NPC, OUT_SIZE], f32, kind="ExternalOutput")
    t_idx = [t_idx0, t_idx1]
    t_slot = [t_slot0, t_slot1]

    with tile.TileContext(nc) as tc:
        with (
            tc.tile_pool(name="dram", bufs=1, space="DRAM") as dram,
            tc.tile_pool(name="const", bufs=1) as const,
            tc.tile_pool(name="hst", bufs=4) as hstp,
            tc.tile_pool(name="msg", bufs=8) as msgp,
            tc.tile_pool(name="idxp", bufs=8) as idxp,
            tc.tile_pool(name="oh", bufs=4) as ohp,
            tc.tile_pool(name="res", bufs=4) as resp,
            tc.tile_pool(name="psum", bufs=4, space="PSUM") as psp,
        ):
            hs_bounce = dram.tile([NPC_PAD, DOUT], bf16)
            hs_full = dram.tile([NPAD, DOUT], bf16, name="hs_full", addr_space="Shared")

            # constants (live for the whole kernel)
            w_tiles = []
            for kt in range(4):
                wt = const.tile([128, DOUT], bf16, name=f"w{kt}")
                nc.sync.dma_start(wt[:], t_w[kt])
                w_tiles.append(wt)
            bias_t = const.tile([128, DOUT], f32, name="bias_t")
            nc.sync.dma_start(bias_t[:], t_bias[:])
            iota_t = const.tile([128, 128], bf16, name="iota_t")
            nc.sync.dma_start(iota_t[:], t_iota[:])
            dinv_t = const.tile([128, NW], f32, name="dinv_t")
            nc.sync.dma_start(dinv_t[:], t_dinv[:])
            # full slot tables resident in SBUF (DVE reads strided slices)
            slot_c = []
            for v in range(2):
                st = const.tile([128, int(n_chunks[v])], bf16, name=f"slotc{v}")
                nc.sync.dma_start(st[:], t_slot[v][:])
                slot_c.append(st)

            # ---- Phase 1: hs_slice = xs @ Wcat, then AllGather ----
            with tc.tile_pool(name="xkp", bufs=1) as xkp:
                xk = []
                for kt in range(4):
                    xt_ = xkp.tile([128, NPC_PAD], bf16, name=f"xk{kt}")
                    nc.sync.dma_start(xt_[:], t_xsT[kt])
                    xk.append(xt_)
                for g in range(0, NW, 4):
                    gn = min(4, NW - g)
                    hstage = hstp.tile([128, 4, DOUT], bf16, name="hstage")
                    for j in range(gn):
                        nb = g + j
                        ph = psp.tile([128, DOUT], f32, name="ph", tag="ph")
                        for kt in range(4):
                            nc.tensor.matmul(
                                out=ph[:],
                                lhsT=xk[kt][:, nb * 128 : (nb + 1) * 128],
                                rhs=w_tiles[kt][:],
                                start=(kt == 0),
                                stop=(kt == 3),
                            )
                        nc.vector.tensor_copy(hstage[:, j, :], ph[:])
                    nc.sync.dma_start(
                        hs_bounce[:]
                        .rearrange("(a p) d -> p a d", p=128)[:, g : g + gn, :],
                        hstage[:, :gn, :],
                    )
                # hs_bounce rows are node-major: row = a*128 + p
                nc.gpsimd.collective_compute(
                    "AllGather",
                    mybir.AluOpType.bypass,
                    replica_groups=[list(range(NCORES))],
                    ins=[hs_bounce[:].opt()],
                    outs=[hs_full[:].opt()],
                )

            hs_ap = hs_full[:]
            views = [hs_ap[0 : SPLIT + 128, :], hs_ap[SPLIT:NPAD, :]]

            # ---- Phase 2: grouped gathers + one-hot segment-sum ----
            chmax = [
                max(
                    int(chunk_off[v][min(a + G_WIN, NW)] - chunk_off[v][a])
                    for a in range(0, NW, G_WIN)
                )
                for v in range(2)
            ]
            qn = 0
            for g0 in range(0, NW, G_WIN):
                g1 = min(g0 + G_WIN, NW)
                oh = [None, None]
                msgs = [None, None]
                for v in range(2):
                    ch0, ch1 = int(chunk_off[v][g0]), int(chunk_off[v][g1])
                    ch = ch1 - ch0
                    oh[v] = ohp.tile(
                        [128, chmax[v], 128], bf16, name=f"oh{v}", tag=f"oh{v}"
                    )
                    nc.vector.tensor_tensor(
                        out=oh[v][:, :ch, :],
                        in0=slot_c[v][:, ch0:ch1]
                        .unsqueeze(-1)
                        .broadcast_to([128, ch, 128]),
                        in1=iota_t[:].unsqueeze(1).broadcast_to([128, ch, 128]),
                        op=mybir.AluOpType.is_equal,
                    )
                    idx_t = idxp.tile(
                        [128, chmax[v] * 8], i16, name=f"idx{v}", tag=f"idx{v}"
                    )
                    nc.sync.dma_start(idx_t[:, : ch * 8], t_idx[v][:, ch0 * 8 : ch1 * 8])
                    mt = msgp.tile(
                        [128, chmax[v], DOUT], bf16, name=f"msg{v}", tag=f"msg{v}"
                    )
                    nc.gpsimd.dma_gather(
                        out_ap=mt[:, :ch, :],
                        in_ap=views[v],
                        idxs_ap=idx_t[:, : ch * 8],
                        num_idxs=ch * 128,
                        num_idxs_reg=ch * 128,
                        elem_size=DOUT,
                        single_packet=False,
                        queue_num=qn,
                    )
                    qn = (qn + 1) % NQ
                    msgs[v] = mt
                for w in range(g0, g1):
                    po = psp.tile([128, DOUT], f32, name="po", tag="po")
                    nmm = int(kwv[0][w] + kwv[1][w])
                    i = 0
                    for v in range(2):
                        kbase = int(chunk_off[v][w]) - int(chunk_off[v][g0])
                        for k in range(int(kwv[v][w])):
                            nc.tensor.matmul(
                                out=po[:],
                                lhsT=oh[v][:, kbase + k, :],
                                rhs=msgs[v][:, kbase + k, :],
                                start=(i == 0),
                                stop=(i == nmm - 1),
                            )
                            i += 1
                    res = resp.tile([128, DOUT], f32, name="res", tag="res")
                    nc.vector.scalar_tensor_tensor(
                        out=res[:],
                        in0=po[:],
                        scalar=dinv_t[:, w : w + 1],
                        in1=bias_t[:],
                        op0=mybir.AluOpType.mult,
                        op1=mybir.AluOpType.add,
                    )
                    rows = LASTW_ROWS if w == NW - 1 else 128
                    r0 = w * 128
                    nc.sync.dma_start(t_mu[r0 : r0 + rows, :], res[:rows, 0:OUT_SIZE])
                    nc.sync.dma_start(
                        t_ls[r0 : r0 + rows, :], res[:rows, OUT_SIZE:DOUT]
                    )

    nc.compile()
    return nc


def kernel(x, edge_index, W_mu, b_mu, W_logstd, b_logstd):
    _install_ntff_shim()

    x = np.asarray(x, dtype=np.float32)
    prep = _prep_edges(np.asarray(edge_index))

    # fold dinv into x rows; per-core transposed slice [4,128,NPC_PAD]
    xs = (x * prep["dinv"][:, None]).astype(BF16)
    xsT_cores = []
    for c in range(NCORES):
        sl = np.zeros((IN_SIZE, NPC_PAD), BF16)
        sl[:, :NPC] = xs[c * NPC : (c + 1) * NPC].T
        xsT_cores.append(np.ascontiguousarray(sl.reshape(4, 128, NPC_PAD)))

    wcat = np.concatenate(
        [np.asarray(W_mu, np.float32), np.asarray(W_logstd, np.float32)], axis=1
    ).astype(BF16)
    wcat_t = np.ascontiguousarray(wcat.reshape(4, 128, DOUT))
    bias = np.concatenate(
        [np.asarray(b_mu, np.float32), np.asarray(b_logstd, np.float32)]
    ).astype(np.float32)
    bias_rep = np.tile(bias[None, :], (128, 1))
    iota_arr = np.tile(
        np.arange(128, dtype=np.float32).astype(BF16)[None, :], (128, 1)
    )

    nc = _build_program(prep)

    in_maps = []
    for c in range(NCORES):
        in_maps.append(
            {
                "xsT": xsT_cores[c],
                "wcat": wcat_t,
                "bias": bias_rep,
                "iota": iota_arr,
                "idx0": prep["idx_arrs"][0][c],
                "idx1": prep["idx_arrs"][1][c],
                "slot0": prep["slot_arrs"][0][c],
                "slot1": prep["slot_arrs"][1][c],
                "dinv_out": prep["dinv_out"][c],
            }
        )

    trace = bool(os.environ.get("K_TRACE"))
    res = run_bass_kernel_spmd(
        nc, in_maps, core_ids=list(range(NCORES)), trace=trace
    )
    global LAST_RESULT
    LAST_RESULT = res
    if trace and res.exec_time_ns is not None:
        print(f"HW exec time: {res.exec_time_ns} ns")
    mu = np.concatenate([res.results[c]["out_mu"] for c in range(NCORES)], axis=0)
    ls = np.concatenate([res.results[c]["out_ls"] for c in range(NCORES)], axis=0)
    return (mu, ls)
